# revision 20
# baseline (speedup 1.0000x reference)
"""MinGRU layer Trainium2 kernel (bf16 data-movement edition).

Math (per batch b):
    g = x @ Wg + bg ; v = x @ Wv + bv ; d = x @ Wd + bd
    xs = sigmoid(g) * tanh(v) ; a = 0.001 + 0.998 * sigmoid(d)
    h_t = a_t * h_{t-1} + xs_t  (h_0 = 0, scan over time S)

Sharding: 8 cores = 4 batches x 2 halves of the 1024 output features.
Each core computes h^T[e, s] for its (b, e-half) with zero cross-core
communication; the time recurrence runs on-chip via the VectorE
TensorTensorScan instruction (time on the free axis, features on
partitions).

x and W travel as bf16 (halves HBM traffic and startup latency; rel
err ~3e-3 vs the 2e-2 gate); PSUM accumulation and the whole
ACT/DVE/h path stay fp32. Host pre-tiles x per (chunk, partition,
k-tile) and W per (proj, partition, k-tile) so every DMA is fully
contiguous per partition. Startup is gated at half-projection/
half-chunk granularity on dedicated semaphores so the PE starts
after ~1 MB instead of ~6 MB of DMA.
"""

import os
import sys

for _p in ("/opt/trn_rl_repo", "/root/.axon_site/_ro/trn_rl_repo"):
    if os.path.isdir(_p) and _p not in sys.path:
        sys.path.insert(0, _p)

import numpy as np
import ml_dtypes

import concourse.bass as bass
import concourse.mybir as mybir
from concourse import bass_utils

B, S, D = 4, 4096, 1024
E = 512                # output features per core (D / 2)
NCH = 8                # time chunks
SC = S // NCH          # chunk length (512)
KT = D // 128          # contraction tiles (8)
JB = E // 128          # output-feature blocks per core (4)

F32 = mybir.dt.float32
BF16 = mybir.dt.bfloat16
AF = mybir.ActivationFunctionType
OP = mybir.AluOpType

BF16_NP = ml_dtypes.bfloat16


def _build_bass(nch=NCH, mode="full"):
    """Build the Bass program. nch > NCH replays the 8 data chunks multiple
    times (benchmarking only — amortizes host/RPC overhead out of timing).
    mode: 'full' (real kernel), 'pe' (PE+DMA only), 'noscan' (no DVE
    scans / h stores) — ablations for bottleneck attribution.

    Structure per superchunk u (= chunk pair 2u, 2u+1):
      PE: for p(3 proj): for j(4 e-blocks): k-loop with the two chunks'
          matmuls interleaved on banks ps[j][:,0/1] sharing each weight
          tile (same-bank back-to-back accumulation is slow on HW).
      ACT: 12 ops batched by function (sig g, tanh v, sig d; j inner) to
          avoid activation-table reloads; each covers both chunks.
      DVE: gating multiply (one wide op per chunk), then per (chunk, j):
          decay affine a = 0.998*sig(d)+0.001 followed by the scan
          (TensorTensorScan, time on free axis) with carry chaining.
      SP(sync): weight/bias startup loads, then per-(chunk, j) h stores.
      GPSIMD: x chunk loads (double-buffered 2 ahead, 3 slots).
    """
    assert nch % 2 == 0
    nc = bass.Bass("TRN2", target_bir_lowering=False, debug=False, num_devices=8)

    # x pre-tiled per chunk: element (c, p, k, s) = x^T[128k+p, SC*c+s]
    xt_d = nc.dram_tensor("xt", [NCH, 128, KT, SC], BF16, kind="ExternalInput").ap()
    # weights pre-tiled per projection: (q, p, k, e) = W_q[128k+p, e]
    w_d = nc.dram_tensor("w", [3, 128, KT, E], BF16, kind="ExternalInput").ap()
    bias_d = nc.dram_tensor("bias", [128, 3 * JB], F32, kind="ExternalInput").ap()
    ht_d = nc.dram_tensor("ht", [E, S], F32, kind="ExternalOutput").ap()

    from contextlib import ExitStack

    with ExitStack() as ctx:
        block = ctx.enter_context(nc.Block())
        # one semaphore per x slot: exact per-chunk waits, no cumulative-
        # count ordering hazard, so all 8 chunk loads queue at t=0
        sem_xc = [
            ctx.enter_context(nc.semaphore(f"sem_xc{i}")) for i in range(NCH)
        ]
        # startup k-quarter sems: chunks 0/1 (x) and Wg arrive in 2-k-tile
        # pieces so the PE's first k-loop starts after ~0.75 MB of DMA
        sem_xq = [
            ctx.enter_context(nc.semaphore(f"sem_xq{i}")) for i in range(8)
        ]
        sem_wq = [
            ctx.enter_context(nc.semaphore(f"sem_wq{i}")) for i in range(12)
        ]
        sem_b = ctx.enter_context(nc.semaphore("sem_b"))
        sem_pe = ctx.enter_context(nc.semaphore("sem_pe"))
        sem_act = ctx.enter_context(nc.semaphore("sem_act"))
        sem_dve = ctx.enter_context(nc.semaphore("sem_dve"))
        sem_st = ctx.enter_context(nc.semaphore("sem_st"))
        w_sb = ctx.enter_context(nc.sbuf_tensor("w_sb", [128, 3, KT, E], BF16))
        xt_sb = ctx.enter_context(nc.sbuf_tensor("xt_sb", [128, NCH, KT, SC], BF16))
        bias_sb = ctx.enter_context(nc.sbuf_tensor("bias_sb", [128, 3 * JB], F32))
        sig_g = ctx.enter_context(nc.sbuf_tensor("sig_g", [128, 2, JB, SC], F32))
        tanh_v = ctx.enter_context(nc.sbuf_tensor("tanh_v", [128, 2, JB, SC], F32))
        sig_d = ctx.enter_context(nc.sbuf_tensor("sig_d", [128, 2, JB, SC], F32))
        a_t = ctx.enter_context(nc.sbuf_tensor("a_t", [128, 2, JB, SC], F32))
        xs_t = ctx.enter_context(nc.sbuf_tensor("xs_t", [128, 2, JB, SC], F32))
        h_t = ctx.enter_context(nc.sbuf_tensor("h_t", [128, 2, JB, SC], F32))
        ps = []
        for j in range(JB):
            ps_j = ctx.enter_context(nc.psum_tensor(f"ps{j}", [128, 2, SC], F32))
            ps.append(ps_j)

        # h^T viewed as [p, j, s]; row index of ht is e = 128*j + p
        ht_view = ht_d.rearrange("(j p) s -> p j s", p=128)

        nsc = nch // 2

        # PE group counter: groups complete in (u, p, j, t) order
        def grp_done(u, p, j, t):
            return 24 * u + 8 * p + 2 * j + t + 1

        # ACT op counter: 12 ops per superchunk, (func, j) order
        def act_idx(u, p, j):
            return 12 * u + 4 * p + j + 1

        # DVE op counter per superchunk (18 ops): mult t0, mult t1, then
        # for t in (0,1): for j: affine, scan
        def dve_mult_done(c):
            return 18 * (c // 2) + (c % 2) + 1

        def dve_aff_done(u, t, j):
            return 18 * u + 2 + 8 * t + 2 * j + 1

        def dve_scan_done(u, t, j):
            return 18 * u + 2 + 8 * t + 2 * j + 2

        @block.gpsimd
        def _(gpsimd):
            # All NCH chunk slots are SBUF-resident, so the first NCH loads
            # queue unconditionally at t=0 and stream at full DMA bandwidth.
            # Chunks 0/1 arrive in k-quarters (own sem each — exact waits)
            # interleaved c0/c1 to match the PE's m0/m1 consumption order.
            for q in range(4):
                for c in (0, 1):
                    gpsimd.dma_start(
                        xt_sb[:, c, 2 * q : 2 * q + 2, :],
                        xt_d[c][:, 2 * q : 2 * q + 2, :],
                    ).then_inc(sem_xq[4 * c + q], 16)
            for c in range(2, nch):
                if c >= NCH:
                    # slot WAR: chunk c-NCH fully computed (replay only)
                    cp = c - NCH
                    gpsimd.wait_ge(sem_pe, grp_done(cp // 2, 2, 3, cp % 2))
                gpsimd.dma_start(
                    xt_sb[:, c % NCH, :, :],
                    xt_d[c % NCH],
                ).then_inc(sem_xc[c % NCH], 16)

        @block.tensor
        def _(tensor):
            # Warm-up: ~1.5us of dummy matmuls on (garbage) SBUF keep the
            # PE busy through the free-running HAM throttle window while
            # the startup DMA streams, so the real matmuls start at the
            # full K=8/8 rate instead of paying ~2us of half-rate ramp.
            # Sized to end as the first data quarters land; N=128 keeps
            # the overshoot under ~0.1us. PSUM bank (0,0) is reclaimed by
            # the first real accumulation group's start=True.
            for _ in range(12):
                tensor.matmul(
                    ps[0][:, 0, 0:128], w_sb[:, 0, 0, 0:128],
                    xt_sb[:, 0, 0, 0:128], start=True, stop=True,
                )
            for u in range(nsc):
                c0, c1 = 2 * u, 2 * u + 1
                s0, s1 = c0 % NCH, c1 % NCH
                if u > 0:
                    # chunk pair resident. Slots 0/1's startup loads count
                    # on sem_xq, not sem_xc, hence the c//NCH count for
                    # replay reuse of those slots.
                    for c, s in ((c0, s0), (c1, s1)):
                        thr = c // NCH + (1 if s >= 2 else 0)
                        tensor.wait_ge(sem_xc[s], 16 * thr)
                for p in range(3):
                    for j in range(JB):
                        if (u, p) != (0, 0) and mode != "pe":
                            # banks (j, :) were read by the ACT op of the
                            # previous p-block; wait before rewriting
                            pu, pp = (u, p - 1) if p else (u - 1, 2)
                            tensor.wait_ge(sem_act, act_idx(pu, pp, j))
                        for k in range(KT):
                            if u == 0 and j == 0 and k % 2 == 0:
                                # startup ladder: weight k-quarters per
                                # projection; chunk-0/1 x k-quarters at p=0
                                i = k // 2
                                tensor.wait_ge(sem_wq[4 * p + i], 16)
                                if p == 0:
                                    tensor.wait_ge(sem_xq[i], 16)
                                    tensor.wait_ge(sem_xq[4 + i], 16)
                            w_ap = w_sb[:, p, k, 128 * j : 128 * (j + 1)]
                            m0 = tensor.matmul(
                                ps[j][:, 0, :], w_ap, xt_sb[:, s0, k, :],
                                start=(k == 0), stop=(k == KT - 1),
                            )
                            m1 = tensor.matmul(
                                ps[j][:, 1, :], w_ap, xt_sb[:, s1, k, :],
                                start=(k == 0), stop=(k == KT - 1),
                            )
                        # matmuls complete in pc order; one inc covers both
                        m1.then_inc(sem_pe, 2)

        @block.scalar
        def _(scalar):
            if mode == "pe":
                return
            scalar.wait_ge(sem_b, 16)  # biases resident
            specs = [
                (0, AF.Sigmoid, sig_g, 0),
                (1, AF.Tanh, tanh_v, 1),
                (2, AF.Sigmoid, sig_d, 2),
            ]
            for u in range(nsc):
                if u >= 1 and mode == "full":
                    # sig_g/tanh_v slots were read by DVE mults of superchunk u-1
                    scalar.wait_ge(sem_dve, dve_mult_done(2 * u - 1))
                for p, func, out_sb_t, bcol in specs:
                    if p == 2 and u >= 1 and mode == "full":
                        # sig_d slots were read by the DVE affines of u-1
                        scalar.wait_ge(sem_dve, dve_aff_done(u - 1, 1, JB - 1))
                    for j in range(JB):
                        scalar.wait_ge(sem_pe, grp_done(u, p, j, 1))
                        scalar.activation(
                            out_sb_t[:, :, j, :],
                            ps[j][:, :, :],
                            func,
                            bias=bias_sb[:, 3 * j + bcol : 3 * j + bcol + 1],
                        ).then_inc(sem_act, 1)

        @block.vector
        def _(vector):
            if mode == "pe":
                return
            for u in range(nsc):
                for t in range(2):
                    vector.wait_ge(sem_act, act_idx(u, 1, 3))  # tanh all done
                    vector.tensor_tensor(
                        xs_t[:, t, :, :], sig_g[:, t, :, :], tanh_v[:, t, :, :],
                        OP.mult,
                    ).then_inc(sem_dve, 1)
                for t in range(2):
                    c = 2 * u + t
                    if c >= 2 and mode == "full":
                        # h slots (c%2, :) were read by stores of chunk c-2
                        vector.wait_ge(sem_st, 64 * (c - 1))
                    for j in range(JB):
                        # decay affine then scan, per j so the final-chunk
                        # tail is short and stores can start per j-block
                        vector.wait_ge(sem_act, act_idx(u, 2, j))  # sig_d j
                        vector.tensor_scalar(
                            a_t[:, t, j, :], sig_d[:, t, j, :], 0.998, 0.001,
                            op0=OP.mult, op1=OP.add,
                        ).then_inc(sem_dve, 1)
                        if mode != "full":
                            continue
                        init = (
                            0.0 if c == 0
                            else h_t[:, (c - 1) % 2, j, SC - 1 : SC]
                        )
                        vector.tensor_tensor_scan(
                            h_t[:, c % 2, j, :], a_t[:, t, j, :],
                            xs_t[:, t, j, :], init, OP.mult, OP.add,
                        ).then_inc(sem_dve, 1)

        @block.sync
        def _(sync):
            # weights/biases stream eagerly on the otherwise-idle HWDGE
            # queue; separate sems per load keep every wait exact
            for q in range(4):
                sync.dma_start(
                    w_sb[:, 0, 2 * q : 2 * q + 2, :],
                    w_d[0][:, 2 * q : 2 * q + 2, :],
                ).then_inc(sem_wq[q], 16)
            sync.dma_start(bias_sb[:], bias_d).then_inc(sem_b, 16)
            for p in (1, 2):
                for q in range(4):
                    sync.dma_start(
                        w_sb[:, p, 2 * q : 2 * q + 2, :],
                        w_d[p][:, 2 * q : 2 * q + 2, :],
                    ).then_inc(sem_wq[4 * p + q], 16)
            if mode != "full":
                return
            for c in range(nch):
                u, t = divmod(c, 2)
                for j in range(JB):
                    sync.wait_ge(sem_dve, dve_scan_done(u, t, j))
                    sync.dma_start(
                        ht_view[:, j, SC * (c % NCH) : SC * (c % NCH + 1)],
                        h_t[:, c % 2, j, :],
                    ).then_inc(sem_st, 16)

    return nc


_NC_CACHE = None


def _build_in_maps(inputs):
    x = np.asarray(inputs["x"], dtype=np.float32)
    Wg = np.asarray(inputs["Wg"], dtype=np.float32)
    bg = np.asarray(inputs["bg"], dtype=np.float32)
    Wv = np.asarray(inputs["Wv"], dtype=np.float32)
    bv = np.asarray(inputs["bv"], dtype=np.float32)
    Wd = np.asarray(inputs["Wd"], dtype=np.float32)
    bd = np.asarray(inputs["bd"], dtype=np.float32)

    # per-batch x tiles: (c, p, k, s) = x[b, SC*c + s, 128*k + p]
    xts = []
    for b in range(B):
        xb = x[b].astype(BF16_NP)                       # (S, D)
        xt = np.ascontiguousarray(
            xb.reshape(NCH, SC, KT, 128).transpose(0, 3, 2, 1)
        )                                               # (NCH, 128, KT, SC)
        xts.append(xt)

    in_maps = []
    for core in range(8):
        b, eh = divmod(core, 2)
        sl = slice(E * eh, E * (eh + 1))
        w = np.empty((3, 128, KT, E), dtype=BF16_NP)
        for pi, W in enumerate((Wg, Wv, Wd)):
            # (q, p, k, e) = W[128*k + p, sl.start + e]
            w[pi] = W[:, sl].astype(BF16_NP).reshape(KT, 128, E).transpose(1, 0, 2)
        bias = np.empty((128, 3 * JB), dtype=np.float32)
        for pi, barr in enumerate((bg[sl], bv[sl], bd[sl])):
            b4 = barr.reshape(JB, 128)
            for j in range(JB):
                bias[:, 3 * j + pi] = b4[j]
        in_maps.append({"xt": xts[b], "w": np.ascontiguousarray(w), "bias": bias})
    return in_maps


def kernel(**inputs: np.ndarray) -> np.ndarray:
    global _NC_CACHE
    if _NC_CACHE is None:
        _NC_CACHE = _build_bass()
    nc = _NC_CACHE

    in_maps = _build_in_maps(inputs)
    res = bass_utils.run_bass_kernel_spmd(nc, in_maps, core_ids=list(range(8)))

    out = np.empty((B, S, D), dtype=np.float32)
    for core in range(8):
        b, eh = divmod(core, 2)
        out[b, :, E * eh : E * (eh + 1)] = res.results[core]["ht"].T
    return out


# revision 23
# speedup vs baseline: 3.3495x; 3.3495x over previous
"""MinGRU layer Trainium2 kernel (bf16 data-movement edition).

Math (per batch b):
    g = x @ Wg + bg ; v = x @ Wv + bv ; d = x @ Wd + bd
    xs = sigmoid(g) * tanh(v) ; a = 0.001 + 0.998 * sigmoid(d)
    h_t = a_t * h_{t-1} + xs_t  (h_0 = 0, scan over time S)

Sharding: 8 cores = 4 batches x 2 halves of the 1024 output features.
Each core computes h^T[e, s] for its (b, e-half) with zero cross-core
communication; the time recurrence runs on-chip via the VectorE
TensorTensorScan instruction (time on the free axis, features on
partitions).

x and W travel as bf16 (halves HBM traffic and startup latency; rel
err ~3e-3 vs the 2e-2 gate); PSUM accumulation and the whole
ACT/DVE/h path stay fp32. Host pre-tiles x per (chunk, partition,
k-tile) and W per (proj, partition, k-tile) so every DMA is fully
contiguous per partition. Startup is gated at half-projection/
half-chunk granularity on dedicated semaphores so the PE starts
after ~1 MB instead of ~6 MB of DMA.
"""

import os
import sys

for _p in ("/opt/trn_rl_repo", "/root/.axon_site/_ro/trn_rl_repo"):
    if os.path.isdir(_p) and _p not in sys.path:
        sys.path.insert(0, _p)

import numpy as np
import ml_dtypes

import concourse.bass as bass
import concourse.mybir as mybir
from concourse import bass_utils

B, S, D = 4, 4096, 1024
E = 512                # output features per core (D / 2)
NCH = 8                # time chunks
SC = S // NCH          # chunk length (512)
KT = D // 128          # contraction tiles (8)
JB = E // 128          # output-feature blocks per core (4)

F32 = mybir.dt.float32
BF16 = mybir.dt.bfloat16
AF = mybir.ActivationFunctionType
OP = mybir.AluOpType

BF16_NP = ml_dtypes.bfloat16


def _build_bass(nch=NCH, mode="full"):
    """Build the Bass program. nch > NCH replays the 8 data chunks multiple
    times (benchmarking only — amortizes host/RPC overhead out of timing).
    mode: 'full' (real kernel), 'pe' (PE+DMA only), 'noscan' (no DVE
    scans / h stores) — ablations for bottleneck attribution.

    Structure per superchunk u (= chunk pair 2u, 2u+1):
      PE: for p(3 proj): for j(4 e-blocks): k-loop with the two chunks'
          matmuls interleaved on banks ps[j][:,0/1] sharing each weight
          tile (same-bank back-to-back accumulation is slow on HW).
      ACT: 12 ops batched by function (sig g, tanh v, sig d; j inner) to
          avoid activation-table reloads; each covers both chunks.
      DVE: gating multiply (one wide op per chunk), then per (chunk, j):
          decay affine a = 0.998*sig(d)+0.001 followed by the scan
          (TensorTensorScan, time on free axis) with carry chaining.
      SP(sync): weight/bias startup loads, then per-(chunk, j) h stores.
      GPSIMD: x chunk loads (double-buffered 2 ahead, 3 slots).
    """
    assert nch % 2 == 0
    nc = bass.Bass("TRN2", target_bir_lowering=False, debug=False, num_devices=8)

    # x pre-tiled per chunk: element (c, p, k, s) = x^T[128k+p, SC*c+s]
    xt_d = nc.dram_tensor("xt", [NCH, 128, KT, SC], BF16, kind="ExternalInput").ap()
    # weights pre-tiled per projection: (q, p, k, e) = W_q[128k+p, e]
    w_d = nc.dram_tensor("w", [3, 128, KT, E], BF16, kind="ExternalInput").ap()
    bias_d = nc.dram_tensor("bias", [128, 3 * JB], F32, kind="ExternalInput").ap()
    ht_d = nc.dram_tensor("ht", [E, S], F32, kind="ExternalOutput").ap()

    from contextlib import ExitStack

    with ExitStack() as ctx:
        block = ctx.enter_context(nc.Block())
        # one semaphore per x slot: exact per-chunk waits, no cumulative-
        # count ordering hazard, so all 8 chunk loads queue at t=0
        sem_xc = [
            ctx.enter_context(nc.semaphore(f"sem_xc{i}")) for i in range(NCH)
        ]
        # startup k-quarter sems: chunks 0/1 (x) and Wg arrive in 2-k-tile
        # pieces so the PE's first k-loop starts after ~0.75 MB of DMA
        sem_xq = [
            ctx.enter_context(nc.semaphore(f"sem_xq{i}")) for i in range(8)
        ]
        sem_wq = [
            ctx.enter_context(nc.semaphore(f"sem_wq{i}")) for i in range(12)
        ]
        sem_b = ctx.enter_context(nc.semaphore("sem_b"))
        sem_pe = ctx.enter_context(nc.semaphore("sem_pe"))
        sem_act = ctx.enter_context(nc.semaphore("sem_act"))
        sem_dve = ctx.enter_context(nc.semaphore("sem_dve"))
        sem_st = ctx.enter_context(nc.semaphore("sem_st"))
        w_sb = ctx.enter_context(nc.sbuf_tensor("w_sb", [128, 3, KT, E], BF16))
        xt_sb = ctx.enter_context(nc.sbuf_tensor("xt_sb", [128, NCH, KT, SC], BF16))
        bias_sb = ctx.enter_context(nc.sbuf_tensor("bias_sb", [128, 3 * JB], F32))
        sig_g = ctx.enter_context(nc.sbuf_tensor("sig_g", [128, 2, JB, SC], F32))
        tanh_v = ctx.enter_context(nc.sbuf_tensor("tanh_v", [128, 2, JB, SC], F32))
        sig_d = ctx.enter_context(nc.sbuf_tensor("sig_d", [128, 2, JB, SC], F32))
        a_t = ctx.enter_context(nc.sbuf_tensor("a_t", [128, 2, JB, SC], F32))
        xs_t = ctx.enter_context(nc.sbuf_tensor("xs_t", [128, 2, JB, SC], F32))
        h_t = ctx.enter_context(nc.sbuf_tensor("h_t", [128, 2, JB, SC], F32))
        ps = []
        for j in range(JB):
            ps_j = ctx.enter_context(nc.psum_tensor(f"ps{j}", [128, 2, SC], F32))
            ps.append(ps_j)

        # h^T viewed as [p, j, s]; row index of ht is e = 128*j + p
        ht_view = ht_d.rearrange("(j p) s -> p j s", p=128)

        nsc = nch // 2

        # PE group counter: groups complete in (u, p, j, t) order
        def grp_done(u, p, j, t):
            return 24 * u + 8 * p + 2 * j + t + 1

        # ACT op counter: 12 ops per superchunk, (func, j) order
        def act_idx(u, p, j):
            return 12 * u + 4 * p + j + 1

        # DVE op counter per superchunk (18 ops): mult t0, mult t1, then
        # for j: aff t0, scan t0, aff t1, scan t1. j-major so each j's
        # chain completes as soon as its sig_d ACT op lands — only the
        # final j's chain trails the last matmul of the program.
        def dve_mult_done(c):
            return 18 * (c // 2) + (c % 2) + 1

        def dve_aff_done(u, t, j):
            return 18 * u + 2 + 4 * j + 2 * t + 1

        def dve_scan_done(u, t, j):
            return 18 * u + 2 + 4 * j + 2 * t + 2

        @block.gpsimd
        def _(gpsimd):
            # All NCH chunk slots are SBUF-resident, so the first NCH loads
            # queue unconditionally at t=0 and stream at full DMA bandwidth.
            # Chunks 0/1 arrive in k-quarters (own sem each — exact waits)
            # interleaved c0/c1 to match the PE's m0/m1 consumption order.
            for q in range(4):
                for c in (0, 1):
                    gpsimd.dma_start(
                        xt_sb[:, c, 2 * q : 2 * q + 2, :],
                        xt_d[c][:, 2 * q : 2 * q + 2, :],
                    ).then_inc(sem_xq[4 * c + q], 16)
            for c in range(2, nch):
                if c >= NCH:
                    # slot WAR: chunk c-NCH fully computed (replay only)
                    cp = c - NCH
                    gpsimd.wait_ge(sem_pe, grp_done(cp // 2, 2, 3, cp % 2))
                gpsimd.dma_start(
                    xt_sb[:, c % NCH, :, :],
                    xt_d[c % NCH],
                ).then_inc(sem_xc[c % NCH], 16)

        @block.tensor
        def _(tensor):
            # Warm-up: ~1.5us of dummy matmuls on (garbage) SBUF keep the
            # PE busy through the free-running HAM throttle window while
            # the startup DMA streams, so the real matmuls start at the
            # full K=8/8 rate instead of paying ~2us of half-rate ramp.
            # Sized to end as the first data quarters land; N=128 keeps
            # the overshoot under ~0.1us. PSUM bank (0,0) is reclaimed by
            # the first real accumulation group's start=True.
            for _ in range(12):
                tensor.matmul(
                    ps[0][:, 0, 0:128], w_sb[:, 0, 0, 0:128],
                    xt_sb[:, 0, 0, 0:128], start=True, stop=True,
                )
            for u in range(nsc):
                c0, c1 = 2 * u, 2 * u + 1
                s0, s1 = c0 % NCH, c1 % NCH
                if u > 0:
                    # chunk pair resident. Slots 0/1's startup loads count
                    # on sem_xq, not sem_xc, hence the c//NCH count for
                    # replay reuse of those slots.
                    for c, s in ((c0, s0), (c1, s1)):
                        thr = c // NCH + (1 if s >= 2 else 0)
                        tensor.wait_ge(sem_xc[s], 16 * thr)
                for p in range(3):
                    for j in range(JB):
                        if (u, p) != (0, 0) and mode != "pe":
                            # banks (j, :) were read by the ACT op of the
                            # previous p-block; wait before rewriting
                            pu, pp = (u, p - 1) if p else (u - 1, 2)
                            tensor.wait_ge(sem_act, act_idx(pu, pp, j))
                        for k in range(KT):
                            if u == 0 and j == 0 and k % 2 == 0:
                                # startup ladder: weight k-quarters per
                                # projection; chunk-0/1 x k-quarters at p=0
                                i = k // 2
                                tensor.wait_ge(sem_wq[4 * p + i], 16)
                                if p == 0:
                                    tensor.wait_ge(sem_xq[i], 16)
                                    tensor.wait_ge(sem_xq[4 + i], 16)
                            w_ap = w_sb[:, p, k, 128 * j : 128 * (j + 1)]
                            m0 = tensor.matmul(
                                ps[j][:, 0, :], w_ap, xt_sb[:, s0, k, :],
                                start=(k == 0), stop=(k == KT - 1),
                            )
                            m1 = tensor.matmul(
                                ps[j][:, 1, :], w_ap, xt_sb[:, s1, k, :],
                                start=(k == 0), stop=(k == KT - 1),
                            )
                        # matmuls complete in pc order; one inc covers both
                        m1.then_inc(sem_pe, 2)

        @block.scalar
        def _(scalar):
            if mode == "pe":
                return
            scalar.wait_ge(sem_b, 16)  # biases resident
            specs = [
                (0, AF.Sigmoid, sig_g, 0),
                (1, AF.Tanh, tanh_v, 1),
                (2, AF.Sigmoid, sig_d, 2),
            ]
            for u in range(nsc):
                if u >= 1 and mode == "full":
                    # sig_g/tanh_v slots were read by DVE mults of superchunk u-1
                    scalar.wait_ge(sem_dve, dve_mult_done(2 * u - 1))
                for p, func, out_sb_t, bcol in specs:
                    if p == 2 and u >= 1 and mode == "full":
                        # sig_d slots were read by the DVE affines of u-1
                        scalar.wait_ge(sem_dve, dve_aff_done(u - 1, 1, JB - 1))
                    for j in range(JB):
                        scalar.wait_ge(sem_pe, grp_done(u, p, j, 1))
                        scalar.activation(
                            out_sb_t[:, :, j, :],
                            ps[j][:, :, :],
                            func,
                            bias=bias_sb[:, 3 * j + bcol : 3 * j + bcol + 1],
                        ).then_inc(sem_act, 1)

        @block.vector
        def _(vector):
            if mode == "pe":
                return
            for u in range(nsc):
                for t in range(2):
                    vector.wait_ge(sem_act, act_idx(u, 1, 3))  # tanh all done
                    vector.tensor_tensor(
                        xs_t[:, t, :, :], sig_g[:, t, :, :], tanh_v[:, t, :, :],
                        OP.mult,
                    ).then_inc(sem_dve, 1)
                if u >= 1 and mode == "full":
                    # h slots were read by superchunk u-1's stores
                    vector.wait_ge(sem_st, 128 * u)
                for j in range(JB):
                    # j-major: each j's aff+scan chain (both chunks) fires
                    # as soon as its sig_d ACT op lands, so j0..j2 finish
                    # during the PE's final k-loops and only j3's chain
                    # trails the last matmul
                    vector.wait_ge(sem_act, act_idx(u, 2, j))  # sig_d j
                    for t in range(2):
                        c = 2 * u + t
                        vector.tensor_scalar(
                            a_t[:, t, j, :], sig_d[:, t, j, :], 0.998, 0.001,
                            op0=OP.mult, op1=OP.add,
                        ).then_inc(sem_dve, 1)
                        if mode != "full":
                            continue
                        init = (
                            0.0 if c == 0
                            else h_t[:, (c - 1) % 2, j, SC - 1 : SC]
                        )
                        vector.tensor_tensor_scan(
                            h_t[:, c % 2, j, :], a_t[:, t, j, :],
                            xs_t[:, t, j, :], init, OP.mult, OP.add,
                        ).then_inc(sem_dve, 1)

        @block.sync
        def _(sync):
            # weights/biases stream eagerly on the otherwise-idle HWDGE
            # queue; separate sems per load keep every wait exact
            for q in range(4):
                sync.dma_start(
                    w_sb[:, 0, 2 * q : 2 * q + 2, :],
                    w_d[0][:, 2 * q : 2 * q + 2, :],
                ).then_inc(sem_wq[q], 16)
            sync.dma_start(bias_sb[:], bias_d).then_inc(sem_b, 16)
            for p in (1, 2):
                for q in range(4):
                    sync.dma_start(
                        w_sb[:, p, 2 * q : 2 * q + 2, :],
                        w_d[p][:, 2 * q : 2 * q + 2, :],
                    ).then_inc(sem_wq[4 * p + q], 16)
            if mode != "full":
                return
            for u in range(nsc):
                # j-major to match scan completion order
                for j in range(JB):
                    for t in range(2):
                        c = 2 * u + t
                        sync.wait_ge(sem_dve, dve_scan_done(u, t, j))
                        sync.dma_start(
                            ht_view[:, j, SC * (c % NCH) : SC * (c % NCH + 1)],
                            h_t[:, c % 2, j, :],
                        ).then_inc(sem_st, 16)

    return nc


_NC_CACHE = None


def _build_in_maps(inputs):
    x = np.asarray(inputs["x"], dtype=np.float32)
    Wg = np.asarray(inputs["Wg"], dtype=np.float32)
    bg = np.asarray(inputs["bg"], dtype=np.float32)
    Wv = np.asarray(inputs["Wv"], dtype=np.float32)
    bv = np.asarray(inputs["bv"], dtype=np.float32)
    Wd = np.asarray(inputs["Wd"], dtype=np.float32)
    bd = np.asarray(inputs["bd"], dtype=np.float32)

    # per-batch x tiles: (c, p, k, s) = x[b, SC*c + s, 128*k + p]
    xts = []
    for b in range(B):
        xb = x[b].astype(BF16_NP)                       # (S, D)
        xt = np.ascontiguousarray(
            xb.reshape(NCH, SC, KT, 128).transpose(0, 3, 2, 1)
        )                                               # (NCH, 128, KT, SC)
        xts.append(xt)

    in_maps = []
    for core in range(8):
        b, eh = divmod(core, 2)
        sl = slice(E * eh, E * (eh + 1))
        w = np.empty((3, 128, KT, E), dtype=BF16_NP)
        for pi, W in enumerate((Wg, Wv, Wd)):
            # (q, p, k, e) = W[128*k + p, sl.start + e]
            w[pi] = W[:, sl].astype(BF16_NP).reshape(KT, 128, E).transpose(1, 0, 2)
        bias = np.empty((128, 3 * JB), dtype=np.float32)
        for pi, barr in enumerate((bg[sl], bv[sl], bd[sl])):
            b4 = barr.reshape(JB, 128)
            for j in range(JB):
                bias[:, 3 * j + pi] = b4[j]
        in_maps.append({"xt": xts[b], "w": np.ascontiguousarray(w), "bias": bias})
    return in_maps


def kernel(**inputs: np.ndarray) -> np.ndarray:
    global _NC_CACHE
    if _NC_CACHE is None:
        _NC_CACHE = _build_bass()
    nc = _NC_CACHE

    in_maps = _build_in_maps(inputs)
    res = bass_utils.run_bass_kernel_spmd(nc, in_maps, core_ids=list(range(8)))

    out = np.empty((B, S, D), dtype=np.float32)
    for core in range(8):
        b, eh = divmod(core, 2)
        out[b, :, E * eh : E * (eh + 1)] = res.results[core]["ht"].T
    return out
